# revision 1
# baseline (speedup 1.0000x reference)
"""Causal self-attention (B=4, T=2048, C=1024, H=16, D=64) on 8 TRN2 cores.

Sharding: core c handles batch b = c//2 and head-half hh = c%2 (8 heads).
Each core computes the qkv projection for its heads, causal attention, and
a partial output projection (its heads' rows of W_proj). Host sums the two
partials per batch and adds b_proj.

Per-core kernel (matmul operands in bf16 -> 1 cycle/row on the PE; all
accumulation in fp32 PSUM):
  phase 1: xT resident in SBUF; V = x@Wv + bv in [t, d] layout (+ ones
           column so PV also produces softmax row-sums); qkT = Wqk^T @ xT.
  phase 2: per head pair: S^T = K^T-tiles x Q (row-packed K=64 matmuls at
           partition bases 0/64), exp on ScalarE (1/sqrt(D) scale fused),
           causal by skipping upper-triangle s-tiles, narrowing diagonal
           tiles to their valid column range, and one [128,128] triangular
           mask multiply per diagonal tile; PV accumulation (M=65 with the
           row-sum column); normalization via DVE fast reciprocal + K=1
           fp32 broadcast matmul.
  phase 3: out = Y @ Wp from SBUF-resident Y^T.
"""

from contextlib import ExitStack

import ml_dtypes
import numpy as np

import concourse.bass as bass
import concourse.tile as tile
from concourse import bacc, mybir
from concourse.bass_utils import run_bass_kernel_spmd

F32 = mybir.dt.float32
DT = mybir.dt.bfloat16
NPDT = ml_dtypes.bfloat16
EXP = mybir.ActivationFunctionType.Exp

T = 2048        # tokens per core (one batch element)
C = 1024        # embed dim
H = 8           # local heads per core
D = 64          # head dim
P = 128
CT = C // P     # 8 contraction tiles over embed dim
QC = H * D      # 512 q/k/v channels per core
TJN = T // 512  # 4 t-tiles (free dim) for attention
SIN = T // P    # 16 s-tiles

TRACE = False   # set by test.py for profiling runs


def build_program():
    nc = bacc.Bacc("TRN2", target_bir_lowering=False, debug=False)
    xT = nc.dram_tensor("xT", [C, T], DT, kind="ExternalInput").ap()
    wqk = nc.dram_tensor("wqk", [C, 2 * QC], DT, kind="ExternalInput").ap()
    bqk = nc.dram_tensor("bqk", [2 * QC], F32, kind="ExternalInput").ap()
    wv = nc.dram_tensor("wv", [C, QC], DT, kind="ExternalInput").ap()
    bv = nc.dram_tensor("bv", [QC], DT, kind="ExternalInput").ap()
    wp = nc.dram_tensor("wp", [QC, C], DT, kind="ExternalInput").ap()
    trimask = nc.dram_tensor("trimask", [P, P], DT, kind="ExternalInput").ap()
    ones_in = nc.dram_tensor("ones", [P, P], DT, kind="ExternalInput").ap()
    onesf = nc.dram_tensor("onesf", [P, D], mybir.dt.float32r, kind="ExternalInput").ap()
    out = nc.dram_tensor("out", [T, C], F32, kind="ExternalOutput").ap()

    with tile.TileContext(nc) as tc, ExitStack() as persist:
        p_small = persist.enter_context(tc.tile_pool(name="small", bufs=1))
        bqk_sb = p_small.tile([P, CT], F32, tag="bqk")
        nc.sync.dma_start(bqk_sb, bqk.rearrange("(j p) -> p j", p=P))
        bv_sb = p_small.tile([1, QC], DT, tag="bv")
        nc.sync.dma_start(bv_sb, bv[None, :])
        ones_row = p_small.tile([1, P], DT, tag="ones_row")
        nc.sync.dma_start(ones_row, ones_in[0:1, :])
        ones64f = p_small.tile([P, D], mybir.dt.float32r, tag="ones64f")
        nc.sync.dma_start(ones64f, onesf)
        tri_sb = p_small.tile([P, P], DT, tag="tri")
        nc.sync.dma_start(tri_sb, trimask)

        # persistent across phases 1-2
        p_qkt = persist.enter_context(tc.tile_pool(name="qkt", bufs=1))
        p_va = persist.enter_context(tc.tile_pool(name="va", bufs=1))
        qkt = [p_qkt.tile([P, T], DT, tag=f"qkt{i}", name=f"qkt{i}") for i in range(CT)]
        va = [p_va.tile([P, H, D + 1], DT, tag=f"va{i}", name=f"va{i}") for i in range(SIN)]

        # ---------------- merged phases ----------------
        with ExitStack() as ph:
            p_xt = ph.enter_context(tc.tile_pool(name="xt", bufs=1))
            p_wqk = ph.enter_context(tc.tile_pool(name="wqk", bufs=16))
            xt = [p_xt.tile([P, T], DT, tag=f"xt{j}", name=f"xt{j}") for j in range(CT)]
            for j in range(CT):
                nc.sync.dma_start(xt[j][:, 0:T // 2], xT[j * P:(j + 1) * P, 0:T // 2])
            for j in range(CT):
                nc.sync.dma_start(xt[j][:, T // 2:T], xT[j * P:(j + 1) * P, T // 2:T])

            p_ysb = ph.enter_context(tc.tile_pool(name="ysb", bufs=1))
            ysb = [p_ysb.tile([P, T], DT, tag=f"ysb{i}", name=f"ysb{i}")
                   for i in range(QC // P)]
            p_wp = ph.enter_context(tc.tile_pool(name="wp", bufs=1))
            wpt = [p_wp.tile([P, C], DT, tag=f"wp{i}", name=f"wp{i}")
                   for i in range(QC // P)]
            for i in range(QC // P):
                nc.sync.dma_start(wpt[i], wp[i * P:(i + 1) * P, :])
            p_pt = ph.enter_context(tc.tile_pool(name="pt", bufs=4))
            p_sumr = ph.enter_context(tc.tile_pool(name="sumr", bufs=3))
            p_scat = ph.enter_context(tc.tile_pool(name="scat", bufs=3))
            p_rcpr = ph.enter_context(tc.tile_pool(name="rcpr", bufs=4))
            p_yun = ph.enter_context(tc.tile_pool(name="yun", bufs=6))
            p_rsb = ph.enter_context(tc.tile_pool(name="rsb", bufs=3))
            p_yn = ph.enter_context(tc.tile_pool(name="yn", bufs=3))
            p_o = ph.enter_context(tc.tile_pool(name="o", bufs=4))
            p_dn = ph.enter_context(tc.tile_pool(name="dn", bufs=4, space="DRAM"))
            ps_s = ph.enter_context(tc.tile_pool(name="ps_s", bufs=2, space="PSUM"))
            ps_y = ph.enter_context(tc.tile_pool(name="ps_y", bufs=2, space="PSUM"))
            # shared by v-proj, qk-proj, normalization R, and proj outputs
            ps_r = ph.enter_context(tc.tile_pool(name="ps_r", bufs=2, space="PSUM"))

            # V projection: V[t, d] for all 8 heads at once (+bias via K=1 mm)
            with tc.tile_pool(name="wv", bufs=1) as p_wv:
                wv_sb = [p_wv.tile([P, QC], DT, tag=f"wv{j}", name=f"wv{j}") for j in range(CT)]
                for j in range(CT):
                    nc.sync.dma_start(wv_sb[j], wv[j * P:(j + 1) * P, :])
                for tt in range(SIN):
                    pv = ps_r.tile([P, QC], F32, tag="ps_r", name="pv")
                    for j in range(CT):
                        nc.tensor.matmul(
                            pv, lhsT=xt[j][:, tt * P:(tt + 1) * P],
                            rhs=wv_sb[j], start=(j == 0), stop=False)
                    nc.tensor.matmul(pv, lhsT=ones_row, rhs=bv_sb,
                                     start=False, stop=True)
                    nc.vector.tensor_copy(out=va[tt][:, :, 0:D], in_=pv)
                    nc.sync.dma_start(va[tt][:, :, D:D + 1], ones_in[:, 0:H][:, :, None])

            def qkproj(ch):
                wt = [p_wqk.tile([P, P], DT, tag="wqk", name="wqk") for _ in range(CT)]
                for j in range(CT):
                    nc.sync.dma_start(
                        wt[j], wqk[j * P:(j + 1) * P, ch * P:(ch + 1) * P])
                for tjc in range(TJN):
                    pq = ps_r.tile([P, 512], F32, tag="ps_r", name="pq")
                    for j in range(CT):
                        nc.tensor.matmul(
                            pq, lhsT=wt[j],
                            rhs=xt[j][:, tjc * 512:(tjc + 1) * 512],
                            start=(j == 0), stop=(j == CT - 1))
                    nc.vector.tensor_scalar_add(
                        out=qkt[ch][:, tjc * 512:(tjc + 1) * 512],
                        in0=pq, scalar1=bqk_sb[:, ch:ch + 1])

            def norm_batch(hp, tj, rcp_row, yun_a, yun_b):
                ts = slice(tj * 512, (tj + 1) * 512)
                for head, yun in ((0, yun_a), (1, yun_b)):
                    r = ps_r.tile([P, 512], F32, tag="ps_r", name="r")
                    nc.tensor.matmul(
                        r[0:D, :], lhsT=ones64f[D:D + 1, :],
                        rhs=rcp_row[D:D + 1, head * 512:(head + 1) * 512],
                        start=True, stop=True)
                    r_sb = p_rsb.tile([D, 512], F32, tag="rsb", name="r_sb")
                    nc.vector.tensor_copy(r_sb, r[0:D, :])
                    if head == 0:
                        nc.vector.tensor_mul(ysb[hp][0:D, ts], yun, r_sb)
                    else:
                        ynb = p_yn.tile([D, 512], DT, tag="yn", name="ynb")
                        nc.vector.tensor_mul(ynb, yun, r_sb)
                        nc.sync.dma_start(ysb[hp][D:P, ts], ynb)

            def proj_tile(tt, co):
                po = ps_r.tile([P, 512], F32, tag="ps_r", name="po")
                for i in range(QC // P):
                    nc.tensor.matmul(
                        po, lhsT=ysb[i][:, tt * P:(tt + 1) * P],
                        rhs=wpt[i][:, co * 512:(co + 1) * 512],
                        start=(i == 0), stop=(i == QC // P - 1))
                ot = p_o.tile([P, 512], F32, tag="o", name="ot")
                if tt % 2 == 0:
                    nc.vector.tensor_copy(ot, po)
                else:
                    nc.scalar.copy(ot, po)
                nc.sync.dma_start(
                    out[tt * P:(tt + 1) * P, co * 512:(co + 1) * 512], ot)

            pending = []
            projq = []  # ready-to-run proj tiles, popped between si iterations
            sictr = 0
            for tj in range(TJN):
                for hp in range(4):  # head pairs (local heads 2hp, 2hp+1)
                    if tj == 0:
                        qkproj(hp)
                        qkproj(4 + hp)
                    if hp == 1 and tj >= 1:
                        projq += [(tt, co) for tt in range(4 * (tj - 1), 4 * tj)
                                  for co in range(C // 512)]
                    qt, kt = qkt[hp], qkt[4 + hp]
                    nsi = 4 * tj + 4
                    ya = ps_y.tile([D + 1, 512], F32, tag="ps_y")
                    yb = ps_y.tile([D + 1, 512], F32, tag="ps_y")
                    for si in range(nsi):
                        m = si - 4 * tj  # diagonal-band index (>=0 on diag)
                        o = max(m, 0) * P  # first valid column in this block
                        s = ps_s.tile([P, 1024], F32, tag="ps_s")
                        nc.tensor.matmul(
                            s[:, o:512], lhsT=kt[0:D, si * P:(si + 1) * P],
                            rhs=qt[0:D, tj * 512 + o:(tj + 1) * 512],
                            start=True, stop=True)
                        nc.tensor.matmul(
                            s[:, 512 + o:1024], lhsT=kt[D:P, si * P:(si + 1) * P],
                            rhs=qt[D:P, tj * 512 + o:(tj + 1) * 512],
                            start=True, stop=True)
                        pt = p_pt.tile([P, 1024], DT, tag="pt")
                        if m < 0:
                            nc.scalar.activation(pt, s, EXP, scale=0.125)
                        else:
                            # one strided call covers both heads' valid range
                            pt2 = pt.rearrange("p (h w) -> p h w", h=2)
                            s2 = s.rearrange("p (h w) -> p h w", h=2)
                            nc.scalar.activation(pt2[:, :, o:512], s2[:, :, o:512],
                                                 EXP, scale=0.125)
                            nc.vector.tensor_tensor(
                                pt2[:, :, o:o + P], pt2[:, :, o:o + P],
                                tri_sb[:, None, :].to_broadcast((P, 2, P)),
                                mybir.AluOpType.mult)
                        nc.tensor.matmul(
                            ya[:, o:512], lhsT=va[si][:, 2 * hp, :],
                            rhs=pt[:, o:512],
                            start=(si == 0), stop=(si == nsi - 1))
                        nc.tensor.matmul(
                            yb[:, o:512], lhsT=va[si][:, 2 * hp + 1, :],
                            rhs=pt[:, 512 + o:1024],
                            start=(si == 0), stop=(si == nsi - 1))
                        sictr += 1
                        if projq and sictr % 3 == 0:
                            proj_tile(*projq.pop(0))
                    # release Y fast: copy unnormalized Y and the sums row
                    yun_a = p_yun.tile([D, 512], F32, tag="yun", name="yun_a")
                    yun_b = p_yun.tile([D, 512], F32, tag="yun", name="yun_b")
                    nc.vector.tensor_copy(yun_a, ya[0:D, :])
                    nc.vector.tensor_copy(yun_b, yb[0:D, :])
                    sumr = p_sumr.tile([P, 1024], F32, tag="sumr", name="sumr")
                    nc.vector.tensor_copy(sumr[D:D + 1, 0:512], ya[D:D + 1, :])
                    nc.vector.tensor_copy(sumr[D:D + 1, 512:1024], yb[D:D + 1, :])
                    # lane-parallel reciprocal via a DRAM bounce to [128, 8]
                    sums_d = p_dn.tile([1, 1024], F32, tag="sums_d", name="sums_d")
                    nc.sync.dma_start(sums_d, sumr[D:D + 1, :])
                    scat = p_scat.tile([P, 8], F32, tag="scat", name="scat")
                    nc.sync.dma_start(scat, sums_d.rearrange("1 (a b) -> a b", a=P))
                    scatr = p_scat.tile([P, 8], mybir.dt.float32r, tag="scatr",
                                        name="scatr")
                    with nc.allow_low_precision(reason="elementwise recip"):
                        nc.vector.reciprocal(scatr, scat)
                    rcp_d = p_dn.tile([1, 1024], mybir.dt.float32r, tag="rcp_d",
                                      name="rcp_d")
                    nc.sync.dma_start(rcp_d.rearrange("1 (a b) -> a b", a=P), scatr)
                    rcp_row = p_rcpr.tile([P, 1024], mybir.dt.float32r,
                                          tag="rcpr", name="rcp_row")
                    nc.sync.dma_start(rcp_row[D:D + 1, :], rcp_d)
                    pending.append((hp, tj, rcp_row, yun_a, yun_b))
                    if len(pending) >= 2:
                        norm_batch(*pending.pop(0))
            while pending:
                norm_batch(*pending.pop(0))
            while projq:
                proj_tile(*projq.pop(0))
            for tt in range(4 * (TJN - 1), 4 * TJN):
                for co in range(C // 512):
                    proj_tile(tt, co)

    nc.compile()
    return nc


_PROG = None


def _get_prog():
    global _PROG
    if _PROG is None:
        _PROG = build_program()
    return _PROG


_LAST_RESULT = {}


def kernel(x, W_attn, b_attn, W_proj, b_proj):
    x = np.asarray(x, np.float32)
    W_attn = np.asarray(W_attn, np.float32)
    b_attn = np.asarray(b_attn, np.float32)
    W_proj = np.asarray(W_proj, np.float32)
    b_proj = np.asarray(b_proj, np.float32)
    B = x.shape[0]
    nc = _get_prog()
    f = np.arange(P)[None, :]
    p = np.arange(P)[:, None]
    tri = (f >= p).astype(NPDT)
    cvt = lambda a: np.ascontiguousarray(a).astype(NPDT)
    in_maps = []
    for c in range(2 * B):
        b, hh = divmod(c, 2)
        sl = slice(hh * QC, hh * QC + QC)
        in_maps.append({
            "xT": cvt(x[b].T),
            "wqk": cvt(np.concatenate(
                [W_attn[:, sl], W_attn[:, C + hh * QC:C + hh * QC + QC]], axis=1)),
            "bqk": np.ascontiguousarray(np.concatenate(
                [b_attn[sl], b_attn[C + hh * QC:C + hh * QC + QC]])),
            "wv": cvt(W_attn[:, 2 * C + hh * QC:2 * C + hh * QC + QC]),
            "bv": cvt(b_attn[2 * C + hh * QC:2 * C + hh * QC + QC]),
            "wp": cvt(W_proj[hh * QC:hh * QC + QC, :]),
            "trimask": tri,
            "ones": np.ones((P, P), NPDT),
            "onesf": np.ones((P, D), np.float32),
        })
    res = run_bass_kernel_spmd(nc, in_maps, list(range(2 * B)), trace=TRACE)
    _LAST_RESULT["res"] = res
    out = np.empty((B, T, C), np.float32)
    for b in range(B):
        out[b] = res.results[2 * b]["out"] + res.results[2 * b + 1]["out"] + b_proj
    return out



# revision 10
# speedup vs baseline: 1.0829x; 1.0829x over previous
"""Causal self-attention (B=4, T=2048, C=1024, H=16, D=64) on 8 TRN2 cores.

Sharding: core c handles batch b = c//2 and head-half hh = c%2 (8 heads).
Each core computes the qkv projection for its heads, causal attention, and
a partial output projection (its heads' rows of W_proj). Host sums the two
partials per batch (bf16) and adds b_proj.

Per-core kernel (matmul operands in bf16 -> 1 cycle/row on the PE; all
accumulation in fp32 PSUM):
  phase 1: x^T resident in SBUF as 4 quarter tiles [P, (j tok)] (one DMA
           each - the sync engine arms DMAs at ~600ns, so DMA count is
           startup latency); V = x@Wv + bv in [t, d] layout (+ ones column
           via DVE memset so PV also produces softmax row-sums);
           qkT = Wqk^T @ xT.
  phase 2: per head pair: S^T = K^T-tiles x Q (row-packed K=64 matmuls at
           partition bases 0/64 -> concurrent PE row tiles), exp on ScalarE
           (1/sqrt(D) scale fused), causal by skipping upper-triangle
           s-tiles, narrowing diagonal tiles, and one [128,128] triangular
           mask multiply per diagonal tile; PV accumulation (M=65 with the
           row-sum column); normalization: sums row -> K=1 broadcast matmul
           -> lane-parallel reciprocal_approx_fast -> multiply.
  phase 3: out = Y @ Wp from SBUF-resident Y^T, written as bf16 partials.
"""

from contextlib import ExitStack

import ml_dtypes
import numpy as np

import concourse.bass as bass
import concourse.tile as tile
from concourse import bacc, mybir
from concourse.bass_utils import run_bass_kernel_spmd

F32 = mybir.dt.float32
F32R = mybir.dt.float32r
DT = mybir.dt.bfloat16
NPDT = ml_dtypes.bfloat16
EXP = mybir.ActivationFunctionType.Exp

T = 2048        # tokens per core (one batch element)
C = 1024        # embed dim
H = 8           # local heads per core
D = 64          # head dim
P = 128
CT = C // P     # 8 contraction tiles over embed dim
QC = H * D      # 512 q/k/v channels per core
TJN = T // 512  # 4 t-tiles (free dim) for attention
SIN = T // P    # 16 s-tiles

TRACE = False   # set by test.py for profiling runs


def build_program():
    nc = bacc.Bacc("TRN2", target_bir_lowering=False, debug=False)
    xT = nc.dram_tensor("xT", [C, T], DT, kind="ExternalInput").ap()
    wqk = nc.dram_tensor("wqk", [C, 2 * QC], DT, kind="ExternalInput").ap()
    bqk = nc.dram_tensor("bqk", [2 * QC], F32, kind="ExternalInput").ap()
    wv = nc.dram_tensor("wv", [C, QC], DT, kind="ExternalInput").ap()
    bv = nc.dram_tensor("bv", [QC], DT, kind="ExternalInput").ap()
    wp = nc.dram_tensor("wp", [QC, C], DT, kind="ExternalInput").ap()
    trimask = nc.dram_tensor("trimask", [P, P], DT, kind="ExternalInput").ap()
    ones_in = nc.dram_tensor("ones", [P, P], DT, kind="ExternalInput").ap()
    onesf = nc.dram_tensor("onesf", [P, D], F32R, kind="ExternalInput").ap()
    out = nc.dram_tensor("out", [T, C], DT, kind="ExternalOutput").ap()

    with tile.TileContext(nc) as tc, ExitStack() as persist:
        p_small = persist.enter_context(tc.tile_pool(name="small", bufs=1))
        bqk_sb = p_small.tile([P, CT], F32, tag="bqk")
        bv_sb = p_small.tile([1, QC], DT, tag="bv")
        ones_row = p_small.tile([1, P], DT, tag="ones_row")
        ones64f = p_small.tile([P, D], F32R, tag="ones64f")
        tri_sb = p_small.tile([P, P], DT, tag="tri")

        # persistent across phases 1-2
        p_qkt = persist.enter_context(tc.tile_pool(name="qkt", bufs=1))
        p_va = persist.enter_context(tc.tile_pool(name="va", bufs=1))
        qkt = [p_qkt.tile([P, T], DT, tag=f"qkt{i}", name=f"qkt{i}") for i in range(CT)]
        va = [p_va.tile([P, H, D + 1], DT, tag=f"va{i}", name=f"va{i}") for i in range(SIN)]

        # ---------------- merged phases ----------------
        with ExitStack() as ph:
            # x^T as 4 quarter tiles, layout [p, (j, tok512)] - one DMA each
            p_xt = ph.enter_context(tc.tile_pool(name="xt", bufs=1))
            p_wqk = ph.enter_context(tc.tile_pool(name="wqk", bufs=3))
            xtq = [p_xt.tile([P, CT * 512], DT, tag=f"xtq{q}", name=f"xtq{q}")
                   for q in range(4)]

            def xtb(tt):  # [P, 128] token block tt, embed tile j slicer
                q, w = divmod(tt, 4)
                return lambda j: xtq[q][:, j * 512 + w * P: j * 512 + (w + 1) * P]

            p_ysb = ph.enter_context(tc.tile_pool(name="ysb", bufs=1))
            ysb = [p_ysb.tile([P, T], DT, tag=f"ysb{i}", name=f"ysb{i}")
                   for i in range(QC // P)]
            p_wp = ph.enter_context(tc.tile_pool(name="wp", bufs=1))
            wp_all = p_wp.tile([P, 4 * C], DT, tag="wp", name="wp_all")
            p_pt = ph.enter_context(tc.tile_pool(name="pt", bufs=4))
            p_rcpr = ph.enter_context(tc.tile_pool(name="rcpr", bufs=4))
            p_yun = ph.enter_context(tc.tile_pool(name="yun", bufs=6))
            p_rsb = ph.enter_context(tc.tile_pool(name="rsb", bufs=3))
            p_yn = ph.enter_context(tc.tile_pool(name="yn", bufs=3))
            p_o = ph.enter_context(tc.tile_pool(name="o", bufs=4))
            ps_s = ph.enter_context(tc.tile_pool(name="ps_s", bufs=2, space="PSUM"))
            ps_y = ph.enter_context(tc.tile_pool(name="ps_y", bufs=2, space="PSUM"))
            # shared by v-proj, qk-proj, normalization R, and proj outputs
            ps_r = ph.enter_context(tc.tile_pool(name="ps_r", bufs=2, space="PSUM"))

            # V projection: V[t, d] for all 8 heads at once (+bias via K=1 mm)
            with tc.tile_pool(name="wv", bufs=1) as p_wv:
                # DMA issue order = startup latency: V-proj inputs first.
                wv_all = p_wv.tile([P, CT * QC], DT, tag="wv", name="wv_all")
                nc.sync.dma_start(bv_sb, bv[None, :])
                nc.sync.dma_start(ones_row, ones_in[0:1, :])
                nc.sync.dma_start(
                    wv_all.rearrange("p (j v) -> p j v", j=CT),
                    wv.rearrange("(j p) v -> p j v", p=P))
                nc.sync.dma_start(
                    xtq[0].rearrange("p (j t) -> p j t", j=CT),
                    xT.rearrange("(j p) t -> p j t", p=P)[:, :, 0:512])
                nc.sync.dma_start(tri_sb, trimask)
                for q in range(1, 4):
                    nc.sync.dma_start(
                        xtq[q].rearrange("p (j t) -> p j t", j=CT),
                        xT.rearrange("(j p) t -> p j t", p=P)[:, :, q * 512:(q + 1) * 512])
                nc.sync.dma_start(bqk_sb, bqk.rearrange("(j p) -> p j", p=P))
                nc.sync.dma_start(ones64f, onesf)
                nc.sync.dma_start(
                    wp_all.rearrange("p (i c) -> p i c", i=QC // P),
                    wp.rearrange("(i p) c -> p i c", p=P))
                for tt in range(SIN):
                    pv = ps_r.tile([P, QC], F32, tag="ps_r", name="pv")
                    blk = xtb(tt)
                    for j in range(CT):
                        nc.tensor.matmul(
                            pv, lhsT=blk(j),
                            rhs=wv_all[:, j * QC:(j + 1) * QC],
                            start=(j == 0), stop=False)
                    nc.tensor.matmul(pv, lhsT=ones_row, rhs=bv_sb,
                                     start=False, stop=True)
                    nc.vector.tensor_copy(out=va[tt][:, :, 0:D], in_=pv)
                    nc.vector.memset(va[tt][:, :, D:D + 1], 1.0)

            def qkproj(ch):
                wt = p_wqk.tile([P, CT * P], DT, tag="wqk", name="wqk")
                nc.sync.dma_start(
                    wt.rearrange("p (j w) -> p j w", j=CT),
                    wqk.rearrange("(j p) w -> p j w", p=P)[:, :, ch * P:(ch + 1) * P])
                for tjc in range(TJN):
                    pq = ps_r.tile([P, 512], F32, tag="ps_r", name="pq")
                    for j in range(CT):
                        nc.tensor.matmul(
                            pq, lhsT=wt[:, j * P:(j + 1) * P],
                            rhs=xtq[tjc][:, j * 512:(j + 1) * 512],
                            start=(j == 0), stop=(j == CT - 1))
                    nc.vector.tensor_scalar_add(
                        out=qkt[ch][:, tjc * 512:(tjc + 1) * 512],
                        in0=pq, scalar1=bqk_sb[:, ch:ch + 1])

            def norm_batch(hp, tj, sumr, yun_a, yun_b):
                # broadcast the raw sums row to 64 partitions via a K=1
                # matmul, then take the reciprocal lane-parallel (64 lanes)
                # with the fast-approx custom DVE op (~18 bits, plenty).
                ts = slice(tj * 512, (tj + 1) * 512)
                for head, yun in ((0, yun_a), (1, yun_b)):
                    r = ps_r.tile([P, 512], F32, tag="ps_r", name="r")
                    nc.tensor.matmul(
                        r[0:D, :], lhsT=ones64f[D:D + 1, :],
                        rhs=sumr[D:D + 1, head * 512:(head + 1) * 512],
                        start=True, stop=True)
                    r_sb = p_rsb.tile([D, 512], F32, tag="rsb", name="r_sb")
                    nc.vector.reciprocal_approx_fast(r_sb, r[0:D, :])
                    if head == 0:
                        nc.vector.tensor_mul(ysb[hp][0:D, ts], yun, r_sb)
                    else:
                        ynb = p_yn.tile([D, 512], DT, tag="yn", name="ynb")
                        nc.vector.tensor_mul(ynb, yun, r_sb)
                        nc.sync.dma_start(ysb[hp][D:P, ts], ynb)

            # proj output tiles: two 512-col halves share one ot tile and a
            # single output DMA (the sync engine arms DMAs at ~600ns each)
            ot_open = {}

            def proj_half(tt, co):
                if tt not in ot_open:
                    ot_open[tt] = p_o.tile([P, C], DT, tag="o", name="ot")
                ot = ot_open[tt]
                po = ps_r.tile([P, 512], F32, tag="ps_r", name="po")
                for i in range(QC // P):
                    nc.tensor.matmul(
                        po, lhsT=ysb[i][:, tt * P:(tt + 1) * P],
                        rhs=wp_all[:, i * C + co * 512:i * C + (co + 1) * 512],
                        start=(i == 0), stop=(i == QC // P - 1))
                if co == 0:
                    nc.vector.tensor_copy(ot[:, 0:512], po)
                else:
                    nc.scalar.copy(ot[:, 512:1024], po)
                    nc.sync.dma_start(out[tt * P:(tt + 1) * P, :], ot)
                    del ot_open[tt]

            pending = []
            projq = []  # ready-to-run proj halves, popped between si iters
            sictr = 0
            for tj in range(TJN):
                for hp in range(4):  # head pairs (local heads 2hp, 2hp+1)
                    if tj == 0:
                        qkproj(hp)
                        qkproj(4 + hp)
                    if hp == 1 and tj >= 1:
                        projq += [(tt, co) for tt in range(4 * (tj - 1), 4 * tj)
                                  for co in range(2)]
                    qt, kt = qkt[hp], qkt[4 + hp]
                    nsi = 4 * tj + 4
                    ya = ps_y.tile([D + 1, 512], F32, tag="ps_y")
                    yb = ps_y.tile([D + 1, 512], F32, tag="ps_y")
                    last_blk = (tj == TJN - 1 and hp == 3)
                    for si in range(nsi):
                        # pop proj work into the PE stream *before* the S
                        # matmuls at block starts (where the S/exp pipeline
                        # ramp leaves the PE waiting on PSUM turnover);
                        # during the last tj hold a few back for the tail.
                        sictr += 1
                        hold = 2 if tj == TJN - 1 else 0
                        if len(projq) > hold and (si < 4 or sictr % 3 == 0):
                            proj_half(*projq.pop(0))
                        m = si - 4 * tj  # diagonal-band index (>=0 on diag)
                        o = max(m, 0) * P  # first valid column in this block
                        s = ps_s.tile([P, 1024], F32, tag="ps_s")
                        nc.tensor.matmul(
                            s[:, o:512], lhsT=kt[0:D, si * P:(si + 1) * P],
                            rhs=qt[0:D, tj * 512 + o:(tj + 1) * 512],
                            start=True, stop=True)
                        nc.tensor.matmul(
                            s[:, 512 + o:1024], lhsT=kt[D:P, si * P:(si + 1) * P],
                            rhs=qt[D:P, tj * 512 + o:(tj + 1) * 512],
                            start=True, stop=True)
                        pt = p_pt.tile([P, 1024], DT, tag="pt")
                        if m < 0:
                            nc.scalar.activation(pt, s, EXP, scale=0.125)
                        else:
                            # one strided call covers both heads' valid range
                            pt2 = pt.rearrange("p (h w) -> p h w", h=2)
                            s2 = s.rearrange("p (h w) -> p h w", h=2)
                            nc.scalar.activation(pt2[:, :, o:512], s2[:, :, o:512],
                                                 EXP, scale=0.125)
                            nc.vector.tensor_tensor(
                                pt2[:, :, o:o + P], pt2[:, :, o:o + P],
                                tri_sb[:, None, :].to_broadcast((P, 2, P)),
                                mybir.AluOpType.mult)
                        nc.tensor.matmul(
                            ya[:, o:512], lhsT=va[si][:, 2 * hp, :],
                            rhs=pt[:, o:512],
                            start=(si == 0), stop=(si == nsi - 1))
                        nc.tensor.matmul(
                            yb[:, o:512], lhsT=va[si][:, 2 * hp + 1, :],
                            rhs=pt[:, 512 + o:1024],
                            start=(si == 0), stop=(si == nsi - 1))
                    # release Y fast: copy unnormalized Y and the sums row.
                    # On the last block split the chain across Scalar/Vector
                    # so the final norm's critical path is short.
                    yun_a = p_yun.tile([D, 512], F32, tag="yun", name="yun_a")
                    yun_b = p_yun.tile([D, 512], F32, tag="yun", name="yun_b")
                    sumr = p_rcpr.tile([P, 1024], F32R, tag="rcpr", name="sumr")
                    if last_blk:
                        nc.scalar.copy(yun_a, ya[0:D, :])
                        nc.vector.tensor_copy(yun_b, yb[0:D, :])
                        with nc.allow_low_precision(reason="softmax denom f32r"):
                            nc.scalar.copy(sumr[D:D + 1, 0:512], ya[D:D + 1, :])
                            nc.vector.tensor_copy(sumr[D:D + 1, 512:1024],
                                                  yb[D:D + 1, :])
                    else:
                        nc.vector.tensor_copy(yun_a, ya[0:D, :])
                        nc.vector.tensor_copy(yun_b, yb[0:D, :])
                        with nc.allow_low_precision(reason="softmax denom f32r"):
                            nc.vector.tensor_copy(sumr[D:D + 1, 0:512],
                                                  ya[D:D + 1, :])
                            nc.vector.tensor_copy(sumr[D:D + 1, 512:1024],
                                                  yb[D:D + 1, :])
                    pending.append((hp, tj, sumr, yun_a, yun_b))
                    if len(pending) >= 2:
                        norm_batch(*pending.pop(0))
            # tail: fill the PE with held-back proj halves while the last
            # block's sums/reciprocal dependency chain completes
            for _ in range(2):
                if projq:
                    proj_half(*projq.pop(0))
            while pending:
                norm_batch(*pending.pop(0))
            while projq:
                proj_half(*projq.pop(0))
            for tt in range(4 * (TJN - 1), 4 * TJN):
                for co in range(2):
                    proj_half(tt, co)

    nc.compile()
    return nc


_PROG = None


def _get_prog():
    global _PROG
    if _PROG is None:
        _PROG = build_program()
    return _PROG


_LAST_RESULT = {}


def kernel(x, W_attn, b_attn, W_proj, b_proj):
    x = np.asarray(x, np.float32)
    W_attn = np.asarray(W_attn, np.float32)
    b_attn = np.asarray(b_attn, np.float32)
    W_proj = np.asarray(W_proj, np.float32)
    b_proj = np.asarray(b_proj, np.float32)
    B = x.shape[0]
    nc = _get_prog()
    f = np.arange(P)[None, :]
    p = np.arange(P)[:, None]
    tri = (f >= p).astype(NPDT)
    cvt = lambda a: np.ascontiguousarray(a).astype(NPDT)
    in_maps = []
    for c in range(2 * B):
        b, hh = divmod(c, 2)
        sl = slice(hh * QC, hh * QC + QC)
        in_maps.append({
            "xT": cvt(x[b].T),
            "wqk": cvt(np.concatenate(
                [W_attn[:, sl], W_attn[:, C + hh * QC:C + hh * QC + QC]], axis=1)),
            "bqk": np.ascontiguousarray(np.concatenate(
                [b_attn[sl], b_attn[C + hh * QC:C + hh * QC + QC]])),
            "wv": cvt(W_attn[:, 2 * C + hh * QC:2 * C + hh * QC + QC]),
            "bv": cvt(b_attn[2 * C + hh * QC:2 * C + hh * QC + QC]),
            "wp": cvt(W_proj[hh * QC:hh * QC + QC, :]),
            "trimask": tri,
            "ones": np.ones((P, P), NPDT),
            "onesf": np.ones((P, D), np.float32),
        })
    res = run_bass_kernel_spmd(nc, in_maps, list(range(2 * B)), trace=TRACE)
    _LAST_RESULT["res"] = res
    out = np.empty((B, T, C), np.float32)
    for b in range(B):
        out[b] = (res.results[2 * b]["out"].astype(np.float32)
                  + res.results[2 * b + 1]["out"].astype(np.float32) + b_proj)
    return out
